# revision 17
# baseline (speedup 1.0000x reference)
"""Trainium2 Bass kernel for nn_Centerdist (segment variance loss).

Math: for each id k in [0, 1000):
    loss_k = sum_{i: id_i=k} ||x_i - mean_k||^2 / n_k
           = (sumsq_k - ||sums_k||^2 / n_k) / n_k
    loss = sum_k loss_k / n_uniq

Sharding: rows are partitioned across the 8 NeuronCores BY ID RANGE
(core c owns ids [125c, 125c+125)), so every id's rows live on exactly
one core and no cross-core reduction is needed.  Each core then only has
to build a [128, 128] one-hot per 128-row tile (local ids 0..124) and do
two matmul-accumulates per tile:

    psum[128 ids, 0:256]   += one_hot.T @ x       (fp32r, 1 cyc/col)
    psum[128 ids, 256:512] += one_hot.T @ x^2

accumulating per-id sums and per-id per-dim sums of squares.  x stays
fp32 end-to-end (fp32r matmul streams fp32 at full rate for >=256
moving columns); squares come from the otherwise idle ACT engine.  The
host gather lays each shard out [group, partition, slot, D] so every
DMA moves LOAD_T KiB contiguous per partition — below ~4 KiB
descriptors the SDMA engines fall well short of line rate.  The kernel
is HBM-bound: each core streams its ~32 MiB row shard once at
~330 GB/s, within ~10% of the 358 GB/s per-core HBM limit.

Counts come from a host-side bincount over the small ids array; the
final per-id division and mean run on host over the tiny [8, 128, 512]
partial outputs.
"""

import numpy as np

from concourse import bacc, bass, bass_utils, mybir, tile

F32 = mybir.dt.float32
F32R = mybir.dt.float32r

N_FULL = 262144
D = 256
NUM_IDS = 1000
P = 128
N_CORES = 8
IDS_PER_CORE = NUM_IDS // N_CORES  # 125
RW = 2 * D  # rhs width: [x | x^2]
LOAD_T = 5  # tiles per DMA load
DUAL_RING = False  # alternating HWDGE rings measured no better than sync-only
FUSED_RHS = True  # single 512-col matmul over [x | x^2] regions per tile


def build_program(tiles: int, reps: int = 1):
    """Build the per-core Bass program processing `tiles` 128-row tiles.

    reps>1 repeats the whole pass (for slope-based HW timing); the output
    is identical since each rep restarts the PSUM accumulation group.
    """
    nc = bacc.Bacc(
        "TRN2",
        target_bir_lowering=False,
        debug=False,
        num_devices=N_CORES,
    )
    load_t = min(LOAD_T, tiles)
    assert tiles % load_t == 0
    groups = tiles // load_t
    # host supplies the shard pre-arranged [group, partition, slot, D] so
    # each DMA moves load_t*1KiB contiguous bytes per partition
    x_d = nc.dram_tensor("x", [groups, P, load_t, D], F32R, kind="ExternalInput")
    idst_d = nc.dram_tensor("idst", [P, tiles], F32, kind="ExternalInput")
    iota_d = nc.dram_tensor("iota", [P, P], F32, kind="ExternalInput")
    out_d = nc.dram_tensor("out", [P, RW], F32, kind="ExternalOutput")

    with tile.TileContext(nc) as tc:
        with (
            tc.tile_pool(name="const", bufs=1) as cpool,
            tc.tile_pool(name="xp", bufs=6) as xpool,
            tc.tile_pool(name="sqp", bufs=3) as sqpool,
            tc.tile_pool(name="ohp", bufs=3) as ohpool,
            tc.tile_pool(name="psp", bufs=1, space="PSUM") as pspool,
            tc.tile_pool(name="evp", bufs=1) as evpool,
        ):
            iota_t = cpool.tile([P, P], F32, tag="iota")
            nc.sync.dma_start(iota_t[:], iota_d.ap())
            idst_t = cpool.tile([P, tiles], F32, tag="idst")
            nc.sync.dma_start(idst_t[:], idst_d.ap())

            psum = pspool.tile([P, RW], F32, name="ps", tag="ps")
            x_g = x_d.ap()

            def one_tile(t, oh, x_ap, sq_ap, fused_ap):
                nc.scalar.activation(
                    sq_ap, x_ap, mybir.ActivationFunctionType.Square
                )
                nc.vector.tensor_scalar(
                    out=oh[:],
                    in0=iota_t[:],
                    scalar1=idst_t[:, t : t + 1],
                    scalar2=None,
                    op0=mybir.AluOpType.is_equal,
                )
                if fused_ap is not None:
                    nc.tensor.matmul(
                        psum[:],
                        oh[:],
                        fused_ap,
                        start=(t == 0),
                        stop=(t == tiles - 1),
                    )
                else:
                    nc.tensor.matmul(
                        psum[:, 0:D],
                        oh[:],
                        x_ap,
                        start=(t == 0),
                        stop=(t == tiles - 1),
                    )
                    nc.tensor.matmul(
                        psum[:, D:RW],
                        oh[:],
                        sq_ap,
                        start=(t == 0),
                        stop=(t == tiles - 1),
                    )

            def one_pass():
                for tg in range(groups):
                    # alternate the two HWDGE rings (qSPDynamicHW /
                    # qActDynamicHW) so ring handoffs overlap
                    dma_eng = nc.sync if (tg % 2 == 0 or not DUAL_RING) else nc.scalar
                    if FUSED_RHS:
                        # region 0 = x (DMA, contiguous), region 1 = x^2 (ACT);
                        # one 512-col matmul streams both regions per tile
                        xt = xpool.tile(
                            [P, 2, load_t, D], F32R, name="xt", tag="xt"
                        )
                        dma_eng.dma_start(xt[:, 0], x_g[tg])
                        for tt in range(load_t):
                            t = tg * load_t + tt
                            oh = ohpool.tile([P, P], F32R, name="oh", tag="oh")
                            one_tile(
                                t,
                                oh,
                                xt[:, 0, tt, :],
                                xt[:, 1, tt, :],
                                xt[:, :, tt, :],
                            )
                    else:
                        xt = xpool.tile([P, load_t, D], F32R, name="xt", tag="xt")
                        dma_eng.dma_start(xt[:], x_g[tg])
                        for tt in range(load_t):
                            t = tg * load_t + tt
                            sq = sqpool.tile([P, D], F32R, name="sq", tag="sq")
                            oh = ohpool.tile([P, P], F32R, name="oh", tag="oh")
                            one_tile(t, oh, xt[:, tt, :], sq[:], None)

            if reps == 1:
                one_pass()
            else:
                # hardware loop: same ~800-instruction program for any rep
                # count (used for slope timing); each iteration recomputes
                # the identical PSUM accumulation from scratch
                with tc.For_i(0, reps):
                    one_pass()

            ev = evpool.tile([P, RW], F32, name="ev", tag="ev")
            nc.vector.tensor_copy(ev[:], psum[:])
            nc.sync.dma_start(out_d.ap(), ev[:])

    nc.compile()
    return nc


_PROGRAM_CACHE: dict = {}


def _get_program(tiles: int, reps: int = 1):
    key = (tiles, reps, DUAL_RING, LOAD_T, FUSED_RHS)
    if key not in _PROGRAM_CACHE:
        _PROGRAM_CACHE[key] = build_program(tiles, reps)
    return _PROGRAM_CACHE[key]


def make_in_maps(reid_feat: np.ndarray, ids: np.ndarray):
    """Shard rows by id range: core c gets all rows with id//125 == c.

    Rows are gathered per bucket on host (this is the sharding step), and
    every core's shard is zero-padded to the same tile count so the SPMD
    program is identical across cores.  Pad rows carry local id -1, which
    never matches the one-hot iota and thus contributes nothing.
    """
    x = np.asarray(reid_feat, dtype=np.float32)
    ids_np = np.asarray(ids).astype(np.int64)
    valid = ids_np >= 0

    if not valid.all():
        xv = x[valid]
        idv = ids_np[valid]
    else:
        xv = x
        idv = ids_np
    bucket = idv // IDS_PER_CORE
    perm = np.argsort(bucket, kind="stable")
    xs = np.ascontiguousarray(xv[perm])
    lids = (idv[perm] - bucket[perm] * IDS_PER_CORE).astype(np.float32)
    counts_per_core = np.bincount(bucket, minlength=N_CORES)
    offs = np.concatenate([[0], np.cumsum(counts_per_core)])

    max_rows = int(counts_per_core.max())
    tiles = max(1, -(-max_rows // P))
    if tiles % LOAD_T:
        tiles += LOAD_T - tiles % LOAD_T
    ns = tiles * P

    load_t = min(LOAD_T, tiles)
    groups = tiles // load_t
    iota = np.broadcast_to(
        np.arange(P, dtype=np.float32), (P, P)
    ).copy()
    in_maps = []
    for c in range(N_CORES):
        nrows = int(counts_per_core[c])
        xc = np.zeros((ns, D), dtype=np.float32)
        xc[:nrows] = xs[offs[c] : offs[c + 1]]
        # [group, slot, p, d] -> [group, p, slot, d]: partition p's load_t
        # rows land contiguously for wide DMA descriptors
        xc = np.ascontiguousarray(
            xc.reshape(groups, load_t, P, D).transpose(0, 2, 1, 3)
        )
        lc = np.full(ns, -1.0, dtype=np.float32)
        lc[:nrows] = lids[offs[c] : offs[c + 1]]
        # idst[p, t] = local id of row t*128 + p
        idst = np.ascontiguousarray(lc.reshape(tiles, P).T)
        in_maps.append({"x": xc, "idst": idst, "iota": iota})
    return in_maps, tiles, valid


def finalize(parts: np.ndarray, ids: np.ndarray, valid: np.ndarray) -> np.ndarray:
    """Combine per-core partials [cores, P, 2D] into the scalar loss."""
    agg = parts.astype(np.float64)
    sums = agg[:, :IDS_PER_CORE, :D].reshape(NUM_IDS, D)
    sumsq = agg[:, :IDS_PER_CORE, D:].sum(axis=2).reshape(NUM_IDS)
    ids_np = np.asarray(ids).astype(np.int64)
    counts = np.bincount(
        ids_np[valid], minlength=NUM_IDS
    )[:NUM_IDS].astype(np.float64)
    safe_n = np.maximum(counts, 1.0)
    sq_per_id = sumsq - (sums * sums).sum(axis=1) / safe_n
    per_id_loss = np.where(counts > 0, sq_per_id / safe_n, 0.0)
    n_uniq = float((counts > 0).sum()) + (1.0 if (~valid).any() else 0.0)
    return np.array(per_id_loss.sum() / n_uniq, dtype=np.float32)


def run_device(reid_feat, ids, trace: bool = False):
    in_maps, tiles, valid = make_in_maps(reid_feat, ids)
    nc = _get_program(tiles)
    res = bass_utils.run_bass_kernel_spmd(
        nc, in_maps, core_ids=list(range(N_CORES)), trace=trace
    )
    parts = np.stack([res.results[c]["out"] for c in range(N_CORES)])
    return parts, valid, res


class DeviceRunner:
    """Persistent jitted SPMD executor (mirrors bass2jax.run_bass_via_pjrt)
    so a program can be executed many times for timing without re-tracing."""

    def __init__(self, nc, in_maps, chain: int = 1):
        import jax
        from jax.sharding import Mesh, PartitionSpec
        from jax.experimental.shard_map import shard_map
        from concourse import bass2jax, mybir as mb

        bass2jax.install_neuronx_cc_hook()
        partition_name = (
            nc.partition_id_tensor.name if nc.partition_id_tensor else None
        )
        in_names, out_names, out_avals, zero_outs = [], [], [], []
        for alloc in nc.m.functions[0].allocations:
            if not isinstance(alloc, mb.MemoryLocationSet):
                continue
            name = alloc.memorylocations[0].name
            if alloc.kind == "ExternalInput":
                if name != partition_name:
                    in_names.append(name)
            elif alloc.kind == "ExternalOutput":
                shape = tuple(alloc.tensor_shape)
                npdt = np.dtype(mb.dt.np(alloc.dtype))
                out_names.append(name)
                out_avals.append(jax.core.ShapedArray(shape, npdt))
                zero_outs.append(np.zeros(shape, npdt))
        self.out_names = out_names
        n_params = len(in_names)
        n_outs = len(out_avals)
        all_names = list(in_names) + list(out_names)
        if partition_name is not None:
            all_names.append(partition_name)

        def _body(*args):
            ins = list(args[:n_params])
            outs = list(args[n_params:])
            # chain>1 = several dependent NEFF executions per dispatch, so
            # per-dispatch overhead can be sloped away when timing
            for _ in range(chain):
                operands = ins + outs
                if partition_name is not None:
                    operands.append(bass2jax.partition_id_tensor())
                outs = list(
                    bass2jax._bass_exec_p.bind(
                        *operands,
                        out_avals=tuple(out_avals),
                        in_names=tuple(all_names),
                        out_names=tuple(out_names),
                        lowering_input_output_aliases=(),
                        sim_require_finite=True,
                        sim_require_nnan=True,
                        nc=nc,
                    )
                )
            return tuple(outs)

        devices = jax.devices()[:N_CORES]
        mesh = Mesh(np.asarray(devices), ("core",))
        in_specs = (PartitionSpec("core"),) * (n_params + n_outs)
        out_specs = (PartitionSpec("core"),) * n_outs
        self._fn = jax.jit(
            shard_map(
                _body,
                mesh=mesh,
                in_specs=in_specs,
                out_specs=out_specs,
                check_rep=False,
            ),
            keep_unused=True,
        )
        self._jax = jax
        concat_in = [
            np.concatenate([np.asarray(in_maps[c][nm]) for c in range(N_CORES)], axis=0)
            for nm in in_names
        ]
        concat_zeros = [
            np.zeros((N_CORES * z.shape[0], *z.shape[1:]), z.dtype) for z in zero_outs
        ]
        sharding = jax.sharding.NamedSharding(mesh, PartitionSpec("core"))
        self._args = [jax.device_put(a, sharding) for a in concat_in + concat_zeros]
        self.out_shapes = [a.shape for a in out_avals]

    def run_once(self):
        outs = self._fn(*self._args)
        self._jax.block_until_ready(outs)
        return outs

    def results(self):
        outs = self.run_once()
        return [
            {
                nm: np.asarray(outs[i]).reshape(N_CORES, *self.out_shapes[i])[c]
                for i, nm in enumerate(self.out_names)
            }
            for c in range(N_CORES)
        ]

    def time_exec(self, iters: int = 20, warmup: int = 3):
        import time as _time

        for _ in range(warmup):
            self.run_once()
        times = []
        for _ in range(iters):
            t0 = _time.perf_counter()
            self.run_once()
            times.append(_time.perf_counter() - t0)
        return float(np.median(times)), times


def kernel(reid_feat, ids) -> np.ndarray:
    parts, valid, _ = run_device(reid_feat, ids)
    return finalize(parts, np.asarray(ids), valid)


# revision 18
# speedup vs baseline: 1.0049x; 1.0049x over previous
"""Trainium2 Bass kernel for nn_Centerdist (segment variance loss).

Math: for each id k in [0, 1000):
    loss_k = sum_{i: id_i=k} ||x_i - mean_k||^2 / n_k
           = (sumsq_k - ||sums_k||^2 / n_k) / n_k
    loss = sum_k loss_k / n_uniq

Sharding: rows are partitioned across the 8 NeuronCores BY ID RANGE
(core c owns ids [125c, 125c+125)), so every id's rows live on exactly
one core and no cross-core reduction is needed.  Each core then only has
to build a [128, 128] one-hot per 128-row tile (local ids 0..124) and do
two matmul-accumulates per tile:

    psum[128 ids, 0:256]   += one_hot.T @ x       (fp32r, 1 cyc/col)
    psum[128 ids, 256:512] += one_hot.T @ x^2

accumulating per-id sums and per-id per-dim sums of squares.  x stays
fp32 end-to-end (fp32r matmul streams fp32 at full rate for >=256
moving columns); squares come from the otherwise idle ACT engine.  The
host gather lays each shard out [group, partition, slot, D] so every
DMA moves LOAD_T KiB contiguous per partition — below ~4 KiB
descriptors the SDMA engines fall well short of line rate.  The kernel
is HBM-bound: each core streams its ~32 MiB row shard once at
~330 GB/s, within ~10% of the 358 GB/s per-core HBM limit.

Counts come from a host-side bincount over the small ids array; the
final per-id division and mean run on host over the tiny [8, 128, 512]
partial outputs.
"""

import numpy as np

from concourse import bacc, bass, bass_utils, mybir, tile

F32 = mybir.dt.float32
F32R = mybir.dt.float32r

N_FULL = 262144
D = 256
NUM_IDS = 1000
P = 128
N_CORES = 8
IDS_PER_CORE = NUM_IDS // N_CORES  # 125
RW = 2 * D  # rhs width: [x | x^2]
LOAD_T = 5  # tiles per DMA load
DUAL_RING = False  # alternating HWDGE rings measured no better than sync-only
FUSED_RHS = True  # single 512-col matmul over [x | x^2] regions per tile
XBUFS = 6  # x-tile pipeline depth
STAGGERED = False  # staggered For_i semaphore reset instead of hard barrier


def build_program(tiles: int, reps: int = 1):
    """Build the per-core Bass program processing `tiles` 128-row tiles.

    reps>1 repeats the whole pass (for slope-based HW timing); the output
    is identical since each rep restarts the PSUM accumulation group.
    """
    nc = bacc.Bacc(
        "TRN2",
        target_bir_lowering=False,
        debug=False,
        num_devices=N_CORES,
    )
    load_t = min(LOAD_T, tiles)
    assert tiles % load_t == 0
    groups = tiles // load_t
    # host supplies the shard pre-arranged [group, partition, slot, D] so
    # each DMA moves load_t*1KiB contiguous bytes per partition
    x_d = nc.dram_tensor("x", [groups, P, load_t, D], F32R, kind="ExternalInput")
    idst_d = nc.dram_tensor("idst", [P, tiles], F32, kind="ExternalInput")
    iota_d = nc.dram_tensor("iota", [P, P], F32, kind="ExternalInput")
    out_d = nc.dram_tensor("out", [P, RW], F32, kind="ExternalOutput")

    with tile.TileContext(nc) as tc:
        with (
            tc.tile_pool(name="const", bufs=1) as cpool,
            tc.tile_pool(name="xp", bufs=XBUFS) as xpool,
            tc.tile_pool(name="sqp", bufs=3) as sqpool,
            tc.tile_pool(name="ohp", bufs=3) as ohpool,
            tc.tile_pool(name="psp", bufs=1, space="PSUM") as pspool,
            tc.tile_pool(name="evp", bufs=1) as evpool,
        ):
            iota_t = cpool.tile([P, P], F32, tag="iota")
            nc.sync.dma_start(iota_t[:], iota_d.ap())
            idst_t = cpool.tile([P, tiles], F32, tag="idst")
            nc.sync.dma_start(idst_t[:], idst_d.ap())

            psum = pspool.tile([P, RW], F32, name="ps", tag="ps")
            x_g = x_d.ap()

            def one_tile(t, oh, x_ap, sq_ap, fused_ap):
                nc.scalar.activation(
                    sq_ap, x_ap, mybir.ActivationFunctionType.Square
                )
                nc.vector.tensor_scalar(
                    out=oh[:],
                    in0=iota_t[:],
                    scalar1=idst_t[:, t : t + 1],
                    scalar2=None,
                    op0=mybir.AluOpType.is_equal,
                )
                if fused_ap is not None:
                    nc.tensor.matmul(
                        psum[:],
                        oh[:],
                        fused_ap,
                        start=(t == 0),
                        stop=(t == tiles - 1),
                    )
                else:
                    nc.tensor.matmul(
                        psum[:, 0:D],
                        oh[:],
                        x_ap,
                        start=(t == 0),
                        stop=(t == tiles - 1),
                    )
                    nc.tensor.matmul(
                        psum[:, D:RW],
                        oh[:],
                        sq_ap,
                        start=(t == 0),
                        stop=(t == tiles - 1),
                    )

            def one_pass():
                for tg in range(groups):
                    # alternate the two HWDGE rings (qSPDynamicHW /
                    # qActDynamicHW) so ring handoffs overlap
                    dma_eng = nc.sync if (tg % 2 == 0 or not DUAL_RING) else nc.scalar
                    if FUSED_RHS:
                        # region 0 = x (DMA, contiguous), region 1 = x^2 (ACT);
                        # one 512-col matmul streams both regions per tile
                        xt = xpool.tile(
                            [P, 2, load_t, D], F32R, name="xt", tag="xt"
                        )
                        dma_eng.dma_start(xt[:, 0], x_g[tg])
                        for tt in range(load_t):
                            t = tg * load_t + tt
                            oh = ohpool.tile([P, P], F32R, name="oh", tag="oh")
                            one_tile(
                                t,
                                oh,
                                xt[:, 0, tt, :],
                                xt[:, 1, tt, :],
                                xt[:, :, tt, :],
                            )
                    else:
                        xt = xpool.tile([P, load_t, D], F32R, name="xt", tag="xt")
                        dma_eng.dma_start(xt[:], x_g[tg])
                        for tt in range(load_t):
                            t = tg * load_t + tt
                            sq = sqpool.tile([P, D], F32R, name="sq", tag="sq")
                            oh = ohpool.tile([P, P], F32R, name="oh", tag="oh")
                            one_tile(t, oh, xt[:, tt, :], sq[:], None)

            if reps == 1:
                one_pass()
            else:
                # hardware loop: same ~800-instruction program for any rep
                # count (used for slope timing); each iteration recomputes
                # the identical PSUM accumulation from scratch
                with tc.For_i(0, reps, staggered_reset=STAGGERED):
                    one_pass()

            ev = evpool.tile([P, RW], F32, name="ev", tag="ev")
            nc.vector.tensor_copy(ev[:], psum[:])
            nc.sync.dma_start(out_d.ap(), ev[:])

    nc.compile()
    return nc


_PROGRAM_CACHE: dict = {}


def _get_program(tiles: int, reps: int = 1):
    key = (tiles, reps, DUAL_RING, LOAD_T, FUSED_RHS, XBUFS, STAGGERED)
    if key not in _PROGRAM_CACHE:
        _PROGRAM_CACHE[key] = build_program(tiles, reps)
    return _PROGRAM_CACHE[key]


def make_in_maps(reid_feat: np.ndarray, ids: np.ndarray):
    """Shard rows by id range: core c gets all rows with id//125 == c.

    Rows are gathered per bucket on host (this is the sharding step), and
    every core's shard is zero-padded to the same tile count so the SPMD
    program is identical across cores.  Pad rows carry local id -1, which
    never matches the one-hot iota and thus contributes nothing.
    """
    x = np.asarray(reid_feat, dtype=np.float32)
    ids_np = np.asarray(ids).astype(np.int64)
    valid = ids_np >= 0

    if not valid.all():
        xv = x[valid]
        idv = ids_np[valid]
    else:
        xv = x
        idv = ids_np
    bucket = idv // IDS_PER_CORE
    perm = np.argsort(bucket, kind="stable")
    xs = np.ascontiguousarray(xv[perm])
    lids = (idv[perm] - bucket[perm] * IDS_PER_CORE).astype(np.float32)
    counts_per_core = np.bincount(bucket, minlength=N_CORES)
    offs = np.concatenate([[0], np.cumsum(counts_per_core)])

    max_rows = int(counts_per_core.max())
    tiles = max(1, -(-max_rows // P))
    if tiles % LOAD_T:
        tiles += LOAD_T - tiles % LOAD_T
    ns = tiles * P

    load_t = min(LOAD_T, tiles)
    groups = tiles // load_t
    iota = np.broadcast_to(
        np.arange(P, dtype=np.float32), (P, P)
    ).copy()
    in_maps = []
    for c in range(N_CORES):
        nrows = int(counts_per_core[c])
        xc = np.zeros((ns, D), dtype=np.float32)
        xc[:nrows] = xs[offs[c] : offs[c + 1]]
        # [group, slot, p, d] -> [group, p, slot, d]: partition p's load_t
        # rows land contiguously for wide DMA descriptors
        xc = np.ascontiguousarray(
            xc.reshape(groups, load_t, P, D).transpose(0, 2, 1, 3)
        )
        lc = np.full(ns, -1.0, dtype=np.float32)
        lc[:nrows] = lids[offs[c] : offs[c + 1]]
        # idst[p, t] = local id of row t*128 + p
        idst = np.ascontiguousarray(lc.reshape(tiles, P).T)
        in_maps.append({"x": xc, "idst": idst, "iota": iota})
    return in_maps, tiles, valid


def finalize(parts: np.ndarray, ids: np.ndarray, valid: np.ndarray) -> np.ndarray:
    """Combine per-core partials [cores, P, 2D] into the scalar loss."""
    agg = parts.astype(np.float64)
    sums = agg[:, :IDS_PER_CORE, :D].reshape(NUM_IDS, D)
    sumsq = agg[:, :IDS_PER_CORE, D:].sum(axis=2).reshape(NUM_IDS)
    ids_np = np.asarray(ids).astype(np.int64)
    counts = np.bincount(
        ids_np[valid], minlength=NUM_IDS
    )[:NUM_IDS].astype(np.float64)
    safe_n = np.maximum(counts, 1.0)
    sq_per_id = sumsq - (sums * sums).sum(axis=1) / safe_n
    per_id_loss = np.where(counts > 0, sq_per_id / safe_n, 0.0)
    n_uniq = float((counts > 0).sum()) + (1.0 if (~valid).any() else 0.0)
    return np.array(per_id_loss.sum() / n_uniq, dtype=np.float32)


def run_device(reid_feat, ids, trace: bool = False):
    in_maps, tiles, valid = make_in_maps(reid_feat, ids)
    nc = _get_program(tiles)
    res = bass_utils.run_bass_kernel_spmd(
        nc, in_maps, core_ids=list(range(N_CORES)), trace=trace
    )
    parts = np.stack([res.results[c]["out"] for c in range(N_CORES)])
    return parts, valid, res


class DeviceRunner:
    """Persistent jitted SPMD executor (mirrors bass2jax.run_bass_via_pjrt)
    so a program can be executed many times for timing without re-tracing."""

    def __init__(self, nc, in_maps, chain: int = 1):
        import jax
        from jax.sharding import Mesh, PartitionSpec
        from jax.experimental.shard_map import shard_map
        from concourse import bass2jax, mybir as mb

        bass2jax.install_neuronx_cc_hook()
        partition_name = (
            nc.partition_id_tensor.name if nc.partition_id_tensor else None
        )
        in_names, out_names, out_avals, zero_outs = [], [], [], []
        for alloc in nc.m.functions[0].allocations:
            if not isinstance(alloc, mb.MemoryLocationSet):
                continue
            name = alloc.memorylocations[0].name
            if alloc.kind == "ExternalInput":
                if name != partition_name:
                    in_names.append(name)
            elif alloc.kind == "ExternalOutput":
                shape = tuple(alloc.tensor_shape)
                npdt = np.dtype(mb.dt.np(alloc.dtype))
                out_names.append(name)
                out_avals.append(jax.core.ShapedArray(shape, npdt))
                zero_outs.append(np.zeros(shape, npdt))
        self.out_names = out_names
        n_params = len(in_names)
        n_outs = len(out_avals)
        all_names = list(in_names) + list(out_names)
        if partition_name is not None:
            all_names.append(partition_name)

        def _body(*args):
            ins = list(args[:n_params])
            outs = list(args[n_params:])
            # chain>1 = several dependent NEFF executions per dispatch, so
            # per-dispatch overhead can be sloped away when timing
            for _ in range(chain):
                operands = ins + outs
                if partition_name is not None:
                    operands.append(bass2jax.partition_id_tensor())
                outs = list(
                    bass2jax._bass_exec_p.bind(
                        *operands,
                        out_avals=tuple(out_avals),
                        in_names=tuple(all_names),
                        out_names=tuple(out_names),
                        lowering_input_output_aliases=(),
                        sim_require_finite=True,
                        sim_require_nnan=True,
                        nc=nc,
                    )
                )
            return tuple(outs)

        devices = jax.devices()[:N_CORES]
        mesh = Mesh(np.asarray(devices), ("core",))
        in_specs = (PartitionSpec("core"),) * (n_params + n_outs)
        out_specs = (PartitionSpec("core"),) * n_outs
        self._fn = jax.jit(
            shard_map(
                _body,
                mesh=mesh,
                in_specs=in_specs,
                out_specs=out_specs,
                check_rep=False,
            ),
            keep_unused=True,
        )
        self._jax = jax
        concat_in = [
            np.concatenate([np.asarray(in_maps[c][nm]) for c in range(N_CORES)], axis=0)
            for nm in in_names
        ]
        concat_zeros = [
            np.zeros((N_CORES * z.shape[0], *z.shape[1:]), z.dtype) for z in zero_outs
        ]
        sharding = jax.sharding.NamedSharding(mesh, PartitionSpec("core"))
        self._args = [jax.device_put(a, sharding) for a in concat_in + concat_zeros]
        self.out_shapes = [a.shape for a in out_avals]

    def run_once(self):
        outs = self._fn(*self._args)
        self._jax.block_until_ready(outs)
        return outs

    def results(self):
        outs = self.run_once()
        return [
            {
                nm: np.asarray(outs[i]).reshape(N_CORES, *self.out_shapes[i])[c]
                for i, nm in enumerate(self.out_names)
            }
            for c in range(N_CORES)
        ]

    def time_exec(self, iters: int = 20, warmup: int = 3):
        import time as _time

        for _ in range(warmup):
            self.run_once()
        times = []
        for _ in range(iters):
            t0 = _time.perf_counter()
            self.run_once()
            times.append(_time.perf_counter() - t0)
        return float(np.median(times)), times


def kernel(reid_feat, ids) -> np.ndarray:
    parts, valid, _ = run_device(reid_feat, ids)
    return finalize(parts, np.asarray(ids), valid)
